# revision 2
# baseline (speedup 1.0000x reference)
"""MoE grouped linear (DMoELinear) on 8 Trainium2 NeuronCores.

Expert-parallel sharding: tokens are sorted by expert id, so expert e's
tokens form one contiguous slice. Core e receives expert e's tokens
(padded to a uniform capacity C so all cores run one SPMD NEFF), expert
e's weight (pre-transposed to [d_in, d_out]) and bias, and computes
    y_e = (x_e @ W_e.T).bf16 + b_e.bf16
The host does all routing/gather in numpy; the device kernel is a dense
[C,1024]x[1024,2048] bf16 matmul with a bias epilogue.
"""

import numpy as np
import ml_dtypes

N_TOK, D_IN, D_OUT, N_EXP = 8192, 1024, 2048, 8
N_CORES = 8
P = 128
NFREE = 512  # matmul moving free dim (one PSUM bank of f32)

BF16 = ml_dtypes.bfloat16

_nc_cache: dict[int, object] = {}


def _build_bass(C: int):
    """Emit the per-core Bass/Tile kernel for token capacity C (multiple of 128)."""
    import concourse.bass as bass  # noqa: F401  (registers engines)
    import concourse.mybir as mybir
    import concourse.tile as tile
    from concourse import bacc

    dt = mybir.dt
    KT = D_IN // P        # 8 contraction tiles
    MT = C // P           # token tiles
    NT = D_OUT // NFREE   # 4 output column tiles

    nc = bacc.Bacc("TRN2", target_bir_lowering=False)

    xT_d = nc.dram_tensor("xT", [D_IN, C], dt.bfloat16, kind="ExternalInput")
    wT_d = nc.dram_tensor("wT", [D_IN, D_OUT], dt.bfloat16, kind="ExternalInput")
    bias_d = nc.dram_tensor("biasb", [P, D_OUT], dt.bfloat16, kind="ExternalInput")
    y_d = nc.dram_tensor("y", [C, D_OUT], dt.bfloat16, kind="ExternalOutput")

    with tile.TileContext(nc) as tc:
        with (
            tc.tile_pool(name="persist", bufs=1) as ppool,
            tc.tile_pool(name="yout", bufs=3) as ypool,
            tc.tile_pool(name="psum", bufs=8, space="PSUM") as pspool,
        ):
            x_tiles = []
            w_tiles = []
            for ki in range(KT):
                xt = ppool.tile([P, C], dt.bfloat16, name=f"x{ki}", tag=f"x{ki}")
                nc.sync.dma_start(xt[:], xT_d[ki * P:(ki + 1) * P, :])
                x_tiles.append(xt)
                wt = ppool.tile([P, D_OUT], dt.bfloat16, name=f"w{ki}", tag=f"w{ki}")
                nc.sync.dma_start(wt[:], wT_d[ki * P:(ki + 1) * P, :])
                w_tiles.append(wt)
            bt = ppool.tile([P, D_OUT], dt.bfloat16, name="bias", tag="bias")
            nc.sync.dma_start(bt[:], bias_d[:])

            for mi in range(MT):
                psums = [
                    pspool.tile([P, NFREE], dt.float32, name=f"ps{mi}_{ni}", tag="ps")
                    for ni in range(NT)
                ]
                for ki in range(KT):
                    lhsT = x_tiles[ki][:, mi * P:(mi + 1) * P]
                    for ni in range(NT):
                        nc.tensor.matmul(
                            psums[ni][:],
                            lhsT,
                            w_tiles[ki][:, ni * NFREE:(ni + 1) * NFREE],
                            start=(ki == 0),
                            stop=(ki == KT - 1),
                        )
                yt = ypool.tile([P, D_OUT], dt.bfloat16, name="yt", tag="yt")
                for ni in range(NT):
                    ys = yt[:, ni * NFREE:(ni + 1) * NFREE]
                    # cast f32 PSUM -> bf16 (matches reference: gmm output is bf16)
                    nc.any.tensor_copy(out=ys, in_=psums[ni][:])
                    # bf16 + bf16 bias add (matches reference epilogue)
                    nc.vector.tensor_add(
                        out=ys, in0=ys, in1=bt[:, ni * NFREE:(ni + 1) * NFREE]
                    )
                nc.sync.dma_start(y_d[mi * P:(mi + 1) * P, :], yt[:])

    nc.compile()
    return nc


def _run_spmd(in_maps, C, trace=False, trace_cores=None):
    from concourse.bass_utils import run_bass_kernel_spmd

    nc = _nc_cache.get(C)
    if nc is None:
        nc = _build_bass(C)
        _nc_cache[C] = nc
    return run_bass_kernel_spmd(
        nc,
        in_maps,
        core_ids=list(range(N_CORES)),
        trace=trace,
        trace_cores=trace_cores,
    )


def _prepare(x, weight, bias, ids_sorted):
    """Host-side routing: returns (in_maps, C, counts, starts)."""
    x = np.asarray(x)
    weight = np.asarray(weight)
    bias = np.asarray(bias)
    ids = np.asarray(ids_sorted)

    counts = np.bincount(ids, minlength=N_EXP).astype(np.int64)
    starts = np.zeros(N_EXP, dtype=np.int64)
    starts[1:] = np.cumsum(counts)[:-1]
    C = max(P, int(-(-counts.max() // P) * P))  # round up to multiple of 128

    xb = x.astype(BF16)
    in_maps = []
    for e in range(N_EXP):
        n_e = int(counts[e])
        xeT = np.zeros((D_IN, C), dtype=BF16)
        if n_e:
            xeT[:, :n_e] = xb[starts[e]:starts[e] + n_e].T
        weT = np.ascontiguousarray(weight[e].T).astype(BF16)  # [d_in, d_out]
        be = np.broadcast_to(bias[e].astype(BF16)[None, :], (P, D_OUT))
        in_maps.append(
            {
                "xT": np.ascontiguousarray(xeT),
                "wT": weT,
                "biasb": np.ascontiguousarray(be),
            }
        )
    return in_maps, C, counts, starts


def kernel(x, weight, bias, ids_sorted):
    in_maps, C, counts, starts = _prepare(x, weight, bias, ids_sorted)
    res = _run_spmd(in_maps, C)
    out = np.empty((N_TOK, D_OUT), dtype=BF16)
    for e in range(N_EXP):
        n_e = int(counts[e])
        if n_e:
            out[starts[e]:starts[e] + n_e] = res.results[e]["y"][:n_e]
    return out
